# revision 3
# baseline (speedup 1.0000x reference)
"""Trainium2 Bass kernel for CrossModalAttention.

Reference computation (per batch row b, modalities q,k in {0,1,2}):
  qp[m] = x[m] @ Wq[m] + bq[m];  kp[m] = x[m] @ Wk[m] + bk[m]
  scores[q,k] = v[q] . tanh(qp[q] + kp[k])          (k != q)
  alpha = softmax over k (2 off-diagonal entries per q)
  att[q] = sum_k alpha[q,k] * (x[k] @ Wt[q,k] + bt[q,k])
  fused  = LayerNorm(concat_m(x[m] + att[m]); gamma, beta)

Strategy: pure data parallel over the batch across 8 NeuronCores
(8192 rows per core), parameters replicated.  Per core, batch tiles of
128 rows; all matmuls in bf16 (fp32 PSUM accumulation), with x
transposed on-chip via the DMA x-bar so the embedding dim lands on the
partition (contraction) axis.  Biases are folded into the PSUM
accumulation as K=1 rank-1 matmuls.  The softmax over 2 entries is a
sigmoid of the score difference.
"""

import json

import numpy as np

import concourse.bass as bass
import concourse.bass2jax as bass2jax
import concourse.bass_utils as bass_utils
import concourse.mybir as mybir
import concourse.tile as tile
from concourse.bass_utils import run_bass_kernel_spmd

M, E, A = 3, 512, 256
B_FULL = 65536
N_CORES = 8
BC = B_FULL // N_CORES  # 8192 rows per core
P = 128
EC = E // P  # 4 contraction chunks
LN_EPS = 1e-5

F32 = mybir.dt.float32
BF16 = mybir.dt.bfloat16
AL = mybir.AluOpType
AF = mybir.ActivationFunctionType

# For query modality q the two keys, in a fixed order.
K_FIRST = [1, 0, 0]
K_SECOND = [2, 2, 1]

# ---------------------------------------------------------------------------
# The walrus build in this container rejects instructions carrying more than
# one semaphore wait (limit varies by ISA struct; 1 is universally safe).
# Tile's wait-assignment freely emits several.  Legalize the serialized BIR:
# move excess waits onto NoOp instructions inserted just before the offender
# on the same engine — semantically identical (engine streams are in-order).
# ---------------------------------------------------------------------------
_MAX_WAITS = 1
_REAL_ENGINES = {"PE", "DVE", "Activation", "Pool", "SP"}


def _legalize_waits(bir_json) -> bytes:
    d = json.loads(bir_json)
    n_split = 0
    for f in d.get("functions", []):
        for b in f.get("blocks", []):
            insts = b.get("instructions", [])
            out = []
            for inst in insts:
                si = inst.get("sync_info")
                waits = (si or {}).get("on_wait") or []
                if len(waits) > _MAX_WAITS and inst.get("engine") in _REAL_ENGINES:
                    extra = waits[: len(waits) - _MAX_WAITS]
                    si["on_wait"] = waits[len(waits) - _MAX_WAITS :]
                    for j, w in enumerate(extra):
                        n_split += 1
                        out.append(
                            {
                                "debug": inst.get("debug", 0),
                                "engine": inst["engine"],
                                "ins": [],
                                "name": f"{inst['name']}-ws{j}",
                                "opcode": "NoOp",
                                "outs": [],
                                "sync_info": {"on_update": [], "on_wait": [w]},
                            }
                        )
                out.append(inst)
            b["instructions"] = out
    return json.dumps(d).encode()


_orig_compile_bir_kernel = bass_utils.compile_bir_kernel


def _patched_compile_bir_kernel(bir_json, tmpdir, neff_name="file.neff"):
    return _orig_compile_bir_kernel(_legalize_waits(bir_json), tmpdir, neff_name)


if bass_utils.compile_bir_kernel is not _patched_compile_bir_kernel:
    bass_utils.compile_bir_kernel = _patched_compile_bir_kernel
    bass2jax.compile_bir_kernel = _patched_compile_bir_kernel


def _build(bc: int, fast_gb: bool) -> bass.Bass:
    """Build the per-core program for a batch shard of `bc` rows."""
    nt = bc // P
    nc = bass.Bass()

    x_d = nc.dram_tensor("x", [M, bc, E], F32, kind="ExternalInput")
    wq_d = nc.dram_tensor("Wq", [M, E, A], F32, kind="ExternalInput")
    bq_d = nc.dram_tensor("bq", [M, A], F32, kind="ExternalInput")
    wk_d = nc.dram_tensor("Wk", [M, E, A], F32, kind="ExternalInput")
    bk_d = nc.dram_tensor("bk", [M, A], F32, kind="ExternalInput")
    v_d = nc.dram_tensor("v", [M, A], F32, kind="ExternalInput")
    wt_d = nc.dram_tensor("Wt", [M, M, E, E], F32, kind="ExternalInput")
    bt_d = nc.dram_tensor("bt", [M, M, E], F32, kind="ExternalInput")
    g_d = nc.dram_tensor("gamma", [M * E], F32, kind="ExternalInput")
    be_d = nc.dram_tensor("beta", [M * E], F32, kind="ExternalInput")
    out_d = nc.dram_tensor("out", [bc, M * E], F32, kind="ExternalOutput")

    pairs = [(q, K_FIRST[q]) for q in range(M)] + [(q, K_SECOND[q]) for q in range(M)]

    with tile.TileContext(nc) as tc:
        with (
            tc.tile_pool(name="const", bufs=1) as cpool,
            tc.tile_pool(name="xb", bufs=3) as xbpool,
            tc.tile_pool(name="xt", bufs=3) as xtpool,
            tc.tile_pool(name="qkps", bufs=2) as qkpspool,
            tc.tile_pool(name="tin", bufs=3) as tinpool,
            tc.tile_pool(name="tt", bufs=3) as ttpool,
            tc.tile_pool(name="tsc", bufs=2) as tscpool,
            tc.tile_pool(name="small", bufs=2) as smpool,
            tc.tile_pool(name="tmp", bufs=2) as tmppool,
            tc.tile_pool(name="att", bufs=2) as attpool,
            tc.tile_pool(name="sq", bufs=2) as sqpool,
            tc.tile_pool(name="outp", bufs=2) as outpool,
            tc.tile_pool(name="qkp_ps", bufs=3, space=bass.MemorySpace.PSUM) as qkpsum,
            tc.tile_pool(name="y_ps", bufs=4, space=bass.MemorySpace.PSUM) as ypsum,
        ):
            # ---- resident parameters (bf16) ----
            # wqk[m]: per e-chunk c, cols [c*512, c*512+512) = [Wq[m] | Wk[m]] chunk.
            wqk = [
                cpool.tile([P, EC * 2 * A], BF16, name=f"wqk{m}", tag=f"wqk{m}")
                for m in range(M)
            ]
            for m in range(M):
                for c in range(EC):
                    es = slice(c * P, (c + 1) * P)
                    nc.gpsimd.dma_start(
                        wqk[m][:, c * 2 * A : c * 2 * A + A], wq_d[m, es, :]
                    )
                    nc.gpsimd.dma_start(
                        wqk[m][:, c * 2 * A + A : (c + 1) * 2 * A], wk_d[m, es, :]
                    )
            # wt[(q,k)]: per e-chunk c, cols [c*512,(c+1)*512) = Wt[q,k] chunk.
            wt = {}
            for q, k in pairs:
                t = cpool.tile([P, EC * E], BF16, name=f"wt{q}_{k}", tag=f"wt{q}_{k}")
                wt[(q, k)] = t
                for c in range(EC):
                    nc.gpsimd.dma_start(
                        t[:, c * E : (c + 1) * E], wt_d[q, k, c * P : (c + 1) * P, :]
                    )
            # bias rows
            bqk_row = [
                cpool.tile([1, 2 * A], BF16, name=f"bqk{m}", tag=f"bqk{m}")
                for m in range(M)
            ]
            for m in range(M):
                nc.gpsimd.dma_start(
                    bqk_row[m][:1, 0:A], bq_d[m, :].rearrange("(o a) -> o a", o=1)
                )
                nc.gpsimd.dma_start(
                    bqk_row[m][:1, A : 2 * A],
                    bk_d[m, :].rearrange("(o a) -> o a", o=1),
                )
            bt_row = {}
            for q, k in pairs:
                t = cpool.tile([1, E], BF16, name=f"bt{q}_{k}", tag=f"bt{q}_{k}")
                bt_row[(q, k)] = t
                nc.gpsimd.dma_start(
                    t[:1, :], bt_d[q, k, :].rearrange("(o e) -> o e", o=1)
                )
            # ones rows for K=1 rank-1 matmuls (bias add, partition broadcast)
            ones_col = cpool.tile([1, P], BF16, name="ones", tag="ones")
            nc.vector.memset(ones_col[:1, :], 1.0)
            ones_f32 = cpool.tile([1, P], F32, name="onesf", tag="onesf")
            nc.vector.memset(ones_f32[:1, :], 1.0)

            # v replicated across partitions via rank-1 matmul
            v_rep = []
            for q in range(M):
                row = cpool.tile([1, A], BF16, name=f"vrow{q}", tag=f"vrow{q}")
                nc.gpsimd.dma_start(
                    row[:1, :], v_d[q, :].rearrange("(o a) -> o a", o=1)
                )
                ps = ypsum.tile([P, E], F32, name=f"vbc{q}", tag="y")
                nc.tensor.matmul(ps[:, 0:A], ones_col[:1, :], row[:1, :])
                rep = cpool.tile([P, A], BF16, name=f"vrep{q}", tag=f"vrep{q}")
                nc.scalar.copy(rep[:, :], ps[:, 0:A])
                v_rep.append(rep)

            # gamma/beta replicated (general path only)
            if not fast_gb:
                grow = cpool.tile([1, M * E], F32, name="grow", tag="grow")
                nc.gpsimd.dma_start(grow[:1, :], g_d[:].rearrange("(o e) -> o e", o=1))
                brow = cpool.tile([1, M * E], F32, name="brow", tag="brow")
                nc.gpsimd.dma_start(brow[:1, :], be_d[:].rearrange("(o e) -> o e", o=1))
                g_rep = cpool.tile([P, M * E], F32, name="grep", tag="grep")
                b_rep = cpool.tile([P, M * E], F32, name="brep", tag="brep")
                for src, dst in ((grow, g_rep), (brow, b_rep)):
                    for c in range(M):
                        cs = slice(c * E, (c + 1) * E)
                        ps = ypsum.tile([P, E], F32, name=f"gbc{c}", tag="y")
                        nc.tensor.matmul(ps[:, :], ones_f32[:1, :], src[:1, cs])
                        nc.vector.tensor_copy(dst[:, cs], ps[:, :])

            # ---- main loop over batch tiles ----
            for ti in range(nt):
                bs = slice(ti * P, (ti + 1) * P)

                # load x (cast f32->bf16) and transpose each 128x128 block
                xb = []
                xt = []
                for m in range(M):
                    xbm = xbpool.tile([P, E], BF16, name=f"xb{m}", tag=f"xb{m}")
                    nc.gpsimd.dma_start(xbm[:, :], x_d[m, bs, :])
                    xtm = xtpool.tile([P, EC * P], BF16, name=f"xt{m}", tag=f"xt{m}")
                    for c in range(EC):
                        cs = slice(c * P, (c + 1) * P)
                        nc.sync.dma_start_transpose(xtm[:, cs], xbm[:, cs])
                    xb.append(xbm)
                    xt.append(xtm)

                # qp|kp per modality: PSUM [128, 512]
                qkp_s = []
                for m in range(M):
                    ps = qkpsum.tile([P, 2 * A], F32, name="qkp", tag="qkp")
                    for c in range(EC):
                        nc.tensor.matmul(
                            ps[:, :],
                            xt[m][:, c * P : (c + 1) * P],
                            wqk[m][:, c * 2 * A : (c + 1) * 2 * A],
                            start=(c == 0),
                            stop=False,
                        )
                    nc.tensor.matmul(
                        ps[:, :],
                        ones_col[:1, :],
                        bqk_row[m][:1, :],
                        start=False,
                        stop=True,
                    )
                    sb = qkpspool.tile([P, 2 * A], BF16, name=f"qkps{m}", tag=f"qkps{m}")
                    nc.scalar.copy(sb[:, :], ps[:, :])
                    qkp_s.append(sb)

                # scores: s[:, q] first pair, s[:, 3+q] second pair
                s_t = smpool.tile([P, 8], F32, name="scores", tag="scores")
                for idx, (q, k) in enumerate(pairs):
                    tin = tinpool.tile([P, A], BF16, name="tin", tag="tin")
                    nc.vector.tensor_add(
                        tin[:, :], qkp_s[q][:, 0:A], qkp_s[k][:, A : 2 * A]
                    )
                    tth = ttpool.tile([P, A], BF16, name="tt", tag="tt")
                    nc.scalar.activation(tth[:, :], tin[:, :], AF.Tanh)
                    tsc = tscpool.tile([P, A], BF16, name="tsc", tag="tsc")
                    nc.vector.scalar_tensor_tensor(
                        tsc[:, :],
                        tth[:, :],
                        1.0,
                        v_rep[q][:, :],
                        AL.mult,
                        AL.mult,
                        accum_out=s_t[:, idx : idx + 1],
                    )

                # alpha: a1 = sigmoid(s_first - s_second), a2 = 1 - a1
                d_t = smpool.tile([P, 4], F32, name="dsc", tag="dsc")
                nc.vector.tensor_sub(d_t[:, 0:M], s_t[:, 0:M], s_t[:, M : 2 * M])
                a1 = smpool.tile([P, 4], F32, name="a1", tag="a1")
                nc.scalar.activation(a1[:, 0:M], d_t[:, 0:M], AF.Sigmoid)
                a2 = smpool.tile([P, 4], F32, name="a2", tag="a2")
                nc.vector.tensor_scalar(
                    a2[:, 0:M], a1[:, 0:M], -1.0, 1.0, AL.mult, AL.add
                )

                # attended = x + a1*(y1+bt1) + a2*(y2+bt2); rows = per-q row sums
                attended = attpool.tile([P, M * E], F32, name="attended", tag="attended")
                rows = smpool.tile([P, 4], F32, name="rows", tag="rows")
                for q in range(M):
                    ys = []
                    for k in (K_FIRST[q], K_SECOND[q]):
                        ps = ypsum.tile([P, E], F32, name="y", tag="y")
                        for c in range(EC):
                            nc.tensor.matmul(
                                ps[:, :],
                                xt[k][:, c * P : (c + 1) * P],
                                wt[(q, k)][:, c * E : (c + 1) * E],
                                start=(c == 0),
                                stop=False,
                            )
                        nc.tensor.matmul(
                            ps[:, :],
                            ones_col[:1, :],
                            bt_row[(q, k)][:1, :],
                            start=False,
                            stop=True,
                        )
                        ys.append(ps)
                    tmp = tmppool.tile([P, E], F32, name="tmp", tag="tmp")
                    nc.vector.scalar_tensor_tensor(
                        tmp[:, :],
                        ys[0][:, :],
                        a1[:, q : q + 1],
                        xb[q][:, :],
                        AL.mult,
                        AL.add,
                    )
                    nc.vector.scalar_tensor_tensor(
                        attended[:, q * E : (q + 1) * E],
                        ys[1][:, :],
                        a2[:, q : q + 1],
                        tmp[:, :],
                        AL.mult,
                        AL.add,
                        accum_out=rows[:, q : q + 1],
                    )

                # LayerNorm stats
                mu = smpool.tile([P, 1], F32, name="mu", tag="mu")
                nc.vector.tensor_reduce(
                    mu[:, :], rows[:, 0:M], mybir.AxisListType.X, AL.add
                )
                nc.vector.tensor_scalar(
                    mu[:, :], mu[:, :], 1.0 / (M * E), None, AL.mult
                )
                ssq = smpool.tile([P, 4], F32, name="ssq", tag="ssq")
                for q in range(M):
                    sq = sqpool.tile([P, E], F32, name="sq", tag="sq")
                    nc.scalar.activation(
                        sq[:, :],
                        attended[:, q * E : (q + 1) * E],
                        AF.Square,
                        accum_out=ssq[:, q : q + 1],
                    )
                ex2 = smpool.tile([P, 1], F32, name="ex2", tag="ex2")
                nc.vector.tensor_reduce(
                    ex2[:, :], ssq[:, 0:M], mybir.AxisListType.X, AL.add
                )
                nc.vector.tensor_scalar(
                    ex2[:, :], ex2[:, :], 1.0 / (M * E), LN_EPS, AL.mult, AL.add
                )
                mu2 = smpool.tile([P, 1], F32, name="mu2", tag="mu2")
                nc.vector.tensor_mul(mu2[:, :], mu[:, :], mu[:, :])
                varp = smpool.tile([P, 1], F32, name="varp", tag="varp")
                nc.vector.tensor_sub(varp[:, :], ex2[:, :], mu2[:, :])
                sd = smpool.tile([P, 1], F32, name="sd", tag="sd")
                nc.scalar.activation(sd[:, :], varp[:, :], AF.Sqrt)
                rstd = smpool.tile([P, 1], F32, name="rstd", tag="rstd")
                nc.vector.reciprocal(rstd[:, :], sd[:, :])

                out_t = outpool.tile([P, M * E], F32, name="out", tag="out")
                if fast_gb:
                    nc.vector.tensor_scalar(
                        out_t[:, :],
                        attended[:, :],
                        mu[:, 0:1],
                        rstd[:, 0:1],
                        AL.subtract,
                        AL.mult,
                    )
                else:
                    za = outpool.tile([P, M * E], F32, name="za", tag="za")
                    nc.vector.tensor_scalar(
                        za[:, :],
                        attended[:, :],
                        mu[:, 0:1],
                        rstd[:, 0:1],
                        AL.subtract,
                        AL.mult,
                    )
                    gz = outpool.tile([P, M * E], F32, name="gz", tag="gz")
                    nc.vector.tensor_mul(gz[:, :], za[:, :], g_rep[:, :])
                    nc.vector.tensor_add(out_t[:, :], gz[:, :], b_rep[:, :])
                nc.sync.dma_start(out_d[bs, :], out_t[:, :])

    return nc


_PROGRAM_CACHE: dict = {}


def _get_program(bc: int, fast_gb: bool) -> bass.Bass:
    key = (bc, fast_gb)
    if key not in _PROGRAM_CACHE:
        _PROGRAM_CACHE[key] = _build(bc, fast_gb)
    return _PROGRAM_CACHE[key]


def kernel(**inputs) -> np.ndarray:
    ins = {
        k: np.ascontiguousarray(np.asarray(v, dtype=np.float32))
        for k, v in inputs.items()
    }
    x = ins["x"]
    assert x.shape == (M, B_FULL, E), x.shape
    fast_gb = bool(np.all(ins["gamma"] == 1.0) and np.all(ins["beta"] == 0.0))
    nc = _get_program(BC, fast_gb)

    shared = {
        k: ins[k]
        for k in ("Wq", "bq", "Wk", "bk", "v", "Wt", "bt", "gamma", "beta")
    }
    in_maps = []
    for i in range(N_CORES):
        m = dict(shared)
        m["x"] = np.ascontiguousarray(x[:, i * BC : (i + 1) * BC, :])
        in_maps.append(m)

    res = run_bass_kernel_spmd(nc, in_maps, core_ids=list(range(N_CORES)))
    out = np.concatenate([res.results[i]["out"] for i in range(N_CORES)], axis=0)
    return out


if __name__ == "__main__":
    rng = np.random.default_rng(0)
    ins = {
        "x": rng.standard_normal((M, B_FULL, E), dtype=np.float32),
        "Wq": (rng.standard_normal((M, E, A)) / np.sqrt(E)).astype(np.float32),
        "bq": (rng.standard_normal((M, A)) / np.sqrt(E)).astype(np.float32),
        "Wk": (rng.standard_normal((M, E, A)) / np.sqrt(E)).astype(np.float32),
        "bk": (rng.standard_normal((M, A)) / np.sqrt(E)).astype(np.float32),
        "v": (rng.standard_normal((M, A)) / np.sqrt(A)).astype(np.float32),
        "Wt": (rng.standard_normal((M, M, E, E)) / np.sqrt(E)).astype(np.float32),
        "bt": (rng.standard_normal((M, M, E)) / np.sqrt(E)).astype(np.float32),
        "gamma": np.ones((M * E,), np.float32),
        "beta": np.zeros((M * E,), np.float32),
    }
    out = kernel(**ins)
    print("out", out.shape, out.dtype)


# revision 17
# speedup vs baseline: 485.1334x; 485.1334x over previous
"""Trainium2 Bass kernel for CrossModalAttention.

Reference computation (per batch row b, modalities q,k in {0,1,2}):
  qp[m] = x[m] @ Wq[m] + bq[m];  kp[m] = x[m] @ Wk[m] + bk[m]
  scores[q,k] = v[q] . tanh(qp[q] + kp[k])          (k != q)
  alpha = softmax over k (2 off-diagonal entries per q)
  att[q] = sum_k alpha[q,k] * (x[k] @ Wt[q,k] + bt[q,k])
  fused  = LayerNorm(concat_m(x[m] + att[m]); gamma, beta)

Strategy: pure data parallel over the batch across 8 NeuronCores
(8192 rows per core), parameters replicated.  Per core, batch tiles of
128 rows; all matmuls in bf16 (fp32 PSUM accumulation), with x
transposed on-chip via the DMA x-bar so the embedding dim lands on the
partition (contraction) axis.  Biases are folded into the PSUM
accumulation as K=1 rank-1 matmuls.  The softmax over 2 entries is a
sigmoid of the score difference.
"""

import json

import numpy as np

import concourse.bass as bass
import concourse.bass2jax as bass2jax
import concourse.bass_utils as bass_utils
import concourse.mybir as mybir
import concourse.tile as tile
from concourse.bass_utils import run_bass_kernel_spmd

M, E, A = 3, 512, 256
B_FULL = 65536
N_CORES = 8
BC = B_FULL // N_CORES  # 8192 rows per core
P = 128
EC = E // P  # 4 contraction chunks
LN_EPS = 1e-5

F32 = mybir.dt.float32
BF16 = mybir.dt.bfloat16
AL = mybir.AluOpType
AF = mybir.ActivationFunctionType

RSTD_HERON = True  # DVE-only rsqrt (avoids ACT Sqrt table-set reloads)

# For query modality q the two keys, in a fixed order.
K_FIRST = [1, 0, 0]
K_SECOND = [2, 2, 1]

# ---------------------------------------------------------------------------
# The walrus build in this container rejects instructions carrying more than
# one semaphore wait (limit varies by ISA struct; 1 is universally safe).
# Tile's wait-assignment freely emits several.  Legalize the serialized BIR:
# move excess waits onto NoOp instructions inserted just before the offender
# on the same engine — semantically identical (engine streams are in-order).
# ---------------------------------------------------------------------------
_MAX_WAITS = 1
_REAL_ENGINES = {"PE", "DVE", "Activation", "Pool", "SP"}


def _legalize_waits(bir_json) -> bytes:
    d = json.loads(bir_json)
    n_split = 0
    for f in d.get("functions", []):
        for b in f.get("blocks", []):
            insts = b.get("instructions", [])
            out = []
            for inst in insts:
                si = inst.get("sync_info")
                waits = (si or {}).get("on_wait") or []
                if len(waits) > _MAX_WAITS and inst.get("engine") in _REAL_ENGINES:
                    extra = waits[: len(waits) - _MAX_WAITS]
                    si["on_wait"] = waits[len(waits) - _MAX_WAITS :]
                    for j, w in enumerate(extra):
                        n_split += 1
                        out.append(
                            {
                                "debug": inst.get("debug", 0),
                                "engine": inst["engine"],
                                "ins": [],
                                "name": f"{inst['name']}-ws{j}",
                                "opcode": "NoOp",
                                "outs": [],
                                "sync_info": {"on_update": [], "on_wait": [w]},
                            }
                        )
                out.append(inst)
            b["instructions"] = out
    return json.dumps(d).encode()


_orig_compile_bir_kernel = bass_utils.compile_bir_kernel


def _patched_compile_bir_kernel(bir_json, tmpdir, neff_name="file.neff"):
    return _orig_compile_bir_kernel(_legalize_waits(bir_json), tmpdir, neff_name)


if bass_utils.compile_bir_kernel is not _patched_compile_bir_kernel:
    bass_utils.compile_bir_kernel = _patched_compile_bir_kernel
    bass2jax.compile_bir_kernel = _patched_compile_bir_kernel


def _build(bc: int, fast_gb: bool, reps: int = 1) -> bass.Bass:
    """Build the per-core program for a batch shard of `bc` rows.

    `reps` re-runs the whole batch loop that many times via a hardware
    loop — used only for benchmarking (amortizes dispatch overhead).
    """
    nt = bc // P
    nc = bass.Bass()

    x_d = nc.dram_tensor("x", [M, bc, E], F32, kind="ExternalInput")
    g_d = nc.dram_tensor("gamma", [M * E], F32, kind="ExternalInput")
    be_d = nc.dram_tensor("beta", [M * E], F32, kind="ExternalInput")
    # pre-arranged bf16 parameters (prepared host-side in kernel()):
    #   Wqk_bf[m, c] = [Wq[m][c-chunk] | Wk[m][c-chunk]]      [M, EC, P, 2A]
    #   Wt_bf[j, c]  = Wt[q_j, k_j][c-chunk]                  [6, EC, P, E]
    #   bqk_bf[m]    = [bq[m] | bk[m]]                        [M, 2A]
    #   btr_bf[j]    = bt[q_j, k_j]                           [6, E]
    #   v_bf[m]      = v[m]                                   [M, A]
    wqk_d = nc.dram_tensor("Wqk_bf", [M, EC, P, 2 * A], BF16, kind="ExternalInput")
    wtb_d = nc.dram_tensor("Wt_bf", [2 * M, EC, P, E], BF16, kind="ExternalInput")
    bqk_d = nc.dram_tensor("bqk_bf", [M, 2 * A], BF16, kind="ExternalInput")
    btr_d = nc.dram_tensor("btr_bf", [2 * M, E], BF16, kind="ExternalInput")
    vb_d = nc.dram_tensor("v_bf", [M, A], BF16, kind="ExternalInput")
    eye_d = nc.dram_tensor("eye_bf", [P, P], BF16, kind="ExternalInput")
    out_d = nc.dram_tensor("out", [bc, M * E], F32, kind="ExternalOutput")

    pairs = [(q, K_FIRST[q]) for q in range(M)] + [(q, K_SECOND[q]) for q in range(M)]

    with tile.TileContext(nc) as tc:
        with (
            tc.tile_pool(name="const", bufs=1) as cpool,
            tc.tile_pool(name="xb", bufs=3) as xbpool,
            tc.tile_pool(name="xt", bufs=3) as xtpool,
            tc.tile_pool(name="qkps", bufs=2) as qkpspool,
            tc.tile_pool(name="tin", bufs=3) as tinpool,
            tc.tile_pool(name="tt", bufs=3) as ttpool,
            tc.tile_pool(name="tsc", bufs=2) as tscpool,
            tc.tile_pool(name="small", bufs=2) as smpool,
            tc.tile_pool(name="tmp", bufs=2) as tmppool,
            tc.tile_pool(name="att", bufs=2) as attpool,
            tc.tile_pool(name="sq", bufs=2) as sqpool,
            tc.tile_pool(name="outp", bufs=2) as outpool,
            tc.tile_pool(name="qkp_ps", bufs=2, space=bass.MemorySpace.PSUM) as qkpsum,
            tc.tile_pool(name="y_ps", bufs=3, space=bass.MemorySpace.PSUM) as ypsum,
            tc.tile_pool(name="tr_ps", bufs=3, space=bass.MemorySpace.PSUM) as trpsum,
        ):
            # ---- resident parameters (bf16, pre-arranged on host) ----
            # wqk[m]: per e-chunk c, cols [c*512, c*512+512) = [Wq[m] | Wk[m]] chunk.
            wqk = [
                cpool.tile([P, EC * 2 * A], BF16, name=f"wqk{m}", tag=f"wqk{m}")
                for m in range(M)
            ]
            for m in range(M):
                nc.sync.dma_start(
                    wqk[m][:, :].rearrange("p (c a) -> p c a", c=EC),
                    wqk_d[m].rearrange("c p a -> p c a"),
                )
            # wt[(q,k)]: per e-chunk c, cols [c*512,(c+1)*512) = Wt[q,k] chunk.
            wt = {}
            for j, (q, k) in enumerate(pairs):
                t = cpool.tile([P, EC * E], BF16, name=f"wt{q}_{k}", tag=f"wt{q}_{k}")
                wt[(q, k)] = t
                nc.sync.dma_start(
                    t[:, :].rearrange("p (c e) -> p c e", c=EC),
                    wtb_d[j].rearrange("c p e -> p c e"),
                )
            # bias rows
            bqk_row = [
                cpool.tile([1, 2 * A], BF16, name=f"bqk{m}", tag=f"bqk{m}")
                for m in range(M)
            ]
            for m in range(M):
                nc.sync.dma_start(
                    bqk_row[m][:1, :], bqk_d[m, :].rearrange("(o a) -> o a", o=1)
                )
            bt_row = {}
            for j, (q, k) in enumerate(pairs):
                t = cpool.tile([1, E], BF16, name=f"bt{q}_{k}", tag=f"bt{q}_{k}")
                bt_row[(q, k)] = t
                nc.sync.dma_start(
                    t[:1, :], btr_d[j, :].rearrange("(o e) -> o e", o=1)
                )
            # ones rows for K=1 rank-1 matmuls (bias add, partition broadcast)
            ones_col = cpool.tile([1, P], BF16, name="ones", tag="ones")
            nc.vector.memset(ones_col[:1, :], 1.0)
            ones_f32 = cpool.tile([1, P], F32, name="onesf", tag="onesf")
            nc.vector.memset(ones_f32[:1, :], 1.0)
            # identity for PE-mode transposes
            eye = cpool.tile([P, P], BF16, name="eye", tag="eye")
            nc.sync.dma_start(eye[:, :], eye_d[:, :])

            # v replicated across partitions via rank-1 matmul
            v_rep = []
            for q in range(M):
                row = cpool.tile([1, A], BF16, name=f"vrow{q}", tag=f"vrow{q}")
                nc.sync.dma_start(
                    row[:1, :], vb_d[q, :].rearrange("(o a) -> o a", o=1)
                )
                ps = ypsum.tile([P, E], F32, name=f"vbc{q}", tag="y")
                nc.tensor.matmul(ps[:, 0:A], ones_col[:1, :], row[:1, :])
                rep = cpool.tile([P, A], BF16, name=f"vrep{q}", tag=f"vrep{q}")
                nc.scalar.copy(rep[:, :], ps[:, 0:A])
                v_rep.append(rep)

            # gamma/beta replicated (general path only)
            if not fast_gb:
                grow = cpool.tile([1, M * E], F32, name="grow", tag="grow")
                nc.gpsimd.dma_start(grow[:1, :], g_d[:].rearrange("(o e) -> o e", o=1))
                brow = cpool.tile([1, M * E], F32, name="brow", tag="brow")
                nc.gpsimd.dma_start(brow[:1, :], be_d[:].rearrange("(o e) -> o e", o=1))
                g_rep = cpool.tile([P, M * E], F32, name="grep", tag="grep")
                b_rep = cpool.tile([P, M * E], F32, name="brep", tag="brep")
                for src, dst in ((grow, g_rep), (brow, b_rep)):
                    for c in range(M):
                        cs = slice(c * E, (c + 1) * E)
                        ps = ypsum.tile([P, E], F32, name=f"gbc{c}", tag="y")
                        nc.tensor.matmul(ps[:, :], ones_f32[:1, :], src[:1, cs])
                        nc.vector.tensor_copy(dst[:, cs], ps[:, :])

            # ---- main loop over batch tiles ----
            import contextlib

            rep_ctx = (
                tc.For_i(0, reps, 1) if reps > 1 else contextlib.nullcontext()
            )
            with rep_ctx:
                for ti in range(nt):
                    _emit_tile(
                        nc, tc, ti, fast_gb, pairs,
                        x_d, out_d, wqk, wt, bqk_row, bt_row, v_rep, ones_col, eye,
                        g_rep if not fast_gb else None,
                        b_rep if not fast_gb else None,
                        xbpool, xtpool, qkpspool, tinpool, ttpool, tscpool,
                        smpool, tmppool, attpool, sqpool, outpool,
                        qkpsum, ypsum, trpsum,
                    )

    return nc


def _emit_tile(
    nc, tc, ti, fast_gb, pairs,
    x_d, out_d, wqk, wt, bqk_row, bt_row, v_rep, ones_col, eye, g_rep, b_rep,
    xbpool, xtpool, qkpspool, tinpool, ttpool, tscpool,
    smpool, tmppool, attpool, sqpool, outpool, qkpsum, ypsum, trpsum,
):
    if True:  # keep indentation shallow
                bs = slice(ti * P, (ti + 1) * P)

                # load x for all modalities in one cast DMA (f32 -> bf16):
                # xb_all[p, m*E + e] = x[m, bs.start+p, e]
                xb_all = xbpool.tile([P, M * E], BF16, name="xball", tag="xball")
                nc.gpsimd.dma_start(
                    xb_all[:, :].rearrange("p (m e) -> p m e", m=M),
                    x_d[:, bs, :].rearrange("m p e -> p m e"),
                )
                xb = [xb_all[:, m * E : (m + 1) * E] for m in range(M)]
                # transpose each 128x128 block on the PE (via identity),
                # evacuating PSUM -> SBUF on the scalar engine
                xt = []
                for m in range(M):
                    trm = trpsum.tile([P, EC * P], BF16, name=f"tr{m}", tag="tr")
                    for c in range(EC):
                        cs = slice(c * P, (c + 1) * P)
                        nc.tensor.transpose(trm[:, cs], xb[m][:, cs], eye[:, :])
                    xtm = xtpool.tile([P, EC * P], BF16, name=f"xt{m}", tag=f"xt{m}")
                    nc.scalar.copy(xtm[:, :], trm[:, :])
                    xt.append(xtm)

                # qp|kp per modality: PSUM [128, 512]
                qkp_s = []
                for m in range(M):
                    ps = qkpsum.tile([P, 2 * A], F32, name="qkp", tag="qkp")
                    for c in range(EC):
                        nc.tensor.matmul(
                            ps[:, :],
                            xt[m][:, c * P : (c + 1) * P],
                            wqk[m][:, c * 2 * A : (c + 1) * 2 * A],
                            start=(c == 0),
                            stop=False,
                        )
                    nc.tensor.matmul(
                        ps[:, :],
                        ones_col[:1, :],
                        bqk_row[m][:1, :],
                        start=False,
                        stop=True,
                    )
                    sb = qkpspool.tile([P, 2 * A], BF16, name=f"qkps{m}", tag=f"qkps{m}")
                    nc.scalar.copy(sb[:, :], ps[:, :])
                    qkp_s.append(sb)

                # scores: s[:, q] first pair, s[:, 3+q] second pair
                s_t = smpool.tile([P, 8], F32, name="scores", tag="scores")
                tin = tinpool.tile([P, 2 * M * A], BF16, name="tin", tag="tin")
                for idx, (q, k) in enumerate(pairs):
                    nc.vector.tensor_add(
                        tin[:, idx * A : (idx + 1) * A],
                        qkp_s[q][:, 0:A],
                        qkp_s[k][:, A : 2 * A],
                    )
                tth = ttpool.tile([P, 2 * M * A], BF16, name="tt", tag="tt")
                nc.scalar.activation(tth[:, :], tin[:, :], AF.Tanh)
                for idx, (q, k) in enumerate(pairs):
                    tsc = tscpool.tile([P, A], BF16, name="tsc", tag="tsc")
                    nc.vector.scalar_tensor_tensor(
                        tsc[:, :],
                        tth[:, idx * A : (idx + 1) * A],
                        1.0,
                        v_rep[q][:, :],
                        AL.mult,
                        AL.mult,
                        accum_out=s_t[:, idx : idx + 1],
                    )

                # alpha: a1 = sigmoid(s_first - s_second), a2 = 1 - a1
                d_t = smpool.tile([P, 4], F32, name="dsc", tag="dsc")
                nc.vector.tensor_sub(d_t[:, 0:M], s_t[:, 0:M], s_t[:, M : 2 * M])
                a1 = smpool.tile([P, 4], F32, name="a1", tag="a1")
                nc.scalar.activation(a1[:, 0:M], d_t[:, 0:M], AF.Sigmoid)
                a2 = smpool.tile([P, 4], F32, name="a2", tag="a2")
                nc.vector.tensor_scalar(
                    a2[:, 0:M], a1[:, 0:M], -1.0, 1.0, AL.mult, AL.add
                )

                # attended = x + a1*(y1+bt1) + a2*(y2+bt2); rows = per-q row sums
                attended = attpool.tile([P, M * E], F32, name="attended", tag="attended")
                rows = smpool.tile([P, 4], F32, name="rows", tag="rows")
                for q in range(M):
                    ys = []
                    for k in (K_FIRST[q], K_SECOND[q]):
                        ps = ypsum.tile([P, E], F32, name="y", tag="y")
                        for c in range(EC):
                            nc.tensor.matmul(
                                ps[:, :],
                                xt[k][:, c * P : (c + 1) * P],
                                wt[(q, k)][:, c * E : (c + 1) * E],
                                start=(c == 0),
                                stop=False,
                            )
                        nc.tensor.matmul(
                            ps[:, :],
                            ones_col[:1, :],
                            bt_row[(q, k)][:1, :],
                            start=False,
                            stop=True,
                        )
                        ys.append(ps)
                    tmp = tmppool.tile([P, E], F32, name="tmp", tag="tmp")
                    nc.vector.scalar_tensor_tensor(
                        tmp[:, :],
                        ys[0][:, :],
                        a1[:, q : q + 1],
                        xb[q][:, :],
                        AL.mult,
                        AL.add,
                    )
                    nc.vector.scalar_tensor_tensor(
                        attended[:, q * E : (q + 1) * E],
                        ys[1][:, :],
                        a2[:, q : q + 1],
                        tmp[:, :],
                        AL.mult,
                        AL.add,
                        accum_out=rows[:, q : q + 1],
                    )

                # LayerNorm stats
                mu = smpool.tile([P, 1], F32, name="mu", tag="mu")
                nc.vector.tensor_reduce(
                    mu[:, :], rows[:, 0:M], mybir.AxisListType.X, AL.add
                )
                nc.vector.tensor_scalar(
                    mu[:, :], mu[:, :], 1.0 / (M * E), None, AL.mult
                )
                ssq = smpool.tile([P, 4], F32, name="ssq", tag="ssq")
                for q in range(M):
                    sq = sqpool.tile([P, E], F32, name="sq", tag="sq")
                    nc.scalar.activation(
                        sq[:, :],
                        attended[:, q * E : (q + 1) * E],
                        AF.Square,
                        accum_out=ssq[:, q : q + 1],
                    )
                ex2 = smpool.tile([P, 1], F32, name="ex2", tag="ex2")
                nc.vector.tensor_reduce(
                    ex2[:, :], ssq[:, 0:M], mybir.AxisListType.X, AL.add
                )
                nc.vector.tensor_scalar(
                    ex2[:, :], ex2[:, :], 1.0 / (M * E), LN_EPS, AL.mult, AL.add
                )
                mu2 = smpool.tile([P, 1], F32, name="mu2", tag="mu2")
                nc.vector.tensor_mul(mu2[:, :], mu[:, :], mu[:, :])
                varp = smpool.tile([P, 1], F32, name="varp", tag="varp")
                nc.vector.tensor_sub(varp[:, :], ex2[:, :], mu2[:, :])
                if RSTD_HERON:
                    # sqrt via Heron iteration on DVE only: the ACT Sqrt lives
                    # in a different activation-table set than Tanh/Sigmoid,
                    # so using it would force two ~2.7us table reloads per
                    # tile.  s0=(1+v)/2, three Newton steps, then 1/s.
                    sd = smpool.tile([P, 1], F32, name="sd", tag="sd")
                    nc.vector.tensor_scalar(
                        sd[:, :], varp[:, :], 0.5, 0.5, AL.mult, AL.add
                    )
                    for it in range(3):
                        rc = smpool.tile([P, 1], F32, name=f"rc{it}", tag=f"rc{it}")
                        nc.vector.reciprocal(rc[:, :], sd[:, :])
                        tq = smpool.tile([P, 1], F32, name=f"tq{it}", tag=f"tq{it}")
                        nc.vector.tensor_mul(tq[:, :], varp[:, :], rc[:, :])
                        sn = smpool.tile([P, 1], F32, name=f"sn{it}", tag=f"sn{it}")
                        nc.vector.tensor_add(sn[:, :], sd[:, :], tq[:, :])
                        sd2 = smpool.tile([P, 1], F32, name=f"sd{it}", tag=f"sd{it}")
                        nc.vector.tensor_scalar(
                            sd2[:, :], sn[:, :], 0.5, None, AL.mult
                        )
                        sd = sd2
                    rstd = smpool.tile([P, 1], F32, name="rstd", tag="rstd")
                    nc.vector.reciprocal(rstd[:, :], sd[:, :])
                else:
                    sd = smpool.tile([P, 1], F32, name="sd", tag="sd")
                    nc.scalar.activation(sd[:, :], varp[:, :], AF.Sqrt)
                    rstd = smpool.tile([P, 1], F32, name="rstd", tag="rstd")
                    nc.vector.reciprocal(rstd[:, :], sd[:, :])

                out_t = outpool.tile([P, M * E], F32, name="out", tag="out")
                if fast_gb:
                    nc.vector.tensor_scalar(
                        out_t[:, :],
                        attended[:, :],
                        mu[:, 0:1],
                        rstd[:, 0:1],
                        AL.subtract,
                        AL.mult,
                    )
                else:
                    za = outpool.tile([P, M * E], F32, name="za", tag="za")
                    nc.vector.tensor_scalar(
                        za[:, :],
                        attended[:, :],
                        mu[:, 0:1],
                        rstd[:, 0:1],
                        AL.subtract,
                        AL.mult,
                    )
                    gz = outpool.tile([P, M * E], F32, name="gz", tag="gz")
                    nc.vector.tensor_mul(gz[:, :], za[:, :], g_rep[:, :])
                    nc.vector.tensor_add(out_t[:, :], gz[:, :], b_rep[:, :])
                nc.sync.dma_start(out_d[bs, :], out_t[:, :])


_PROGRAM_CACHE: dict = {}


def _get_program(bc: int, fast_gb: bool, reps: int = 1) -> bass.Bass:
    key = (bc, fast_gb, reps)
    if key not in _PROGRAM_CACHE:
        _PROGRAM_CACHE[key] = _build(bc, fast_gb, reps)
    return _PROGRAM_CACHE[key]


def _prep_shared(ins) -> dict:
    """Host-side bf16 pre-arrangement of the (small) parameters."""
    import ml_dtypes

    BFD = ml_dtypes.bfloat16
    pairs = [(q, K_FIRST[q]) for q in range(M)] + [
        (q, K_SECOND[q]) for q in range(M)
    ]
    Wq, Wk, Wt = ins["Wq"], ins["Wk"], ins["Wt"]
    wqk = np.empty((M, EC, P, 2 * A), BFD)
    for m in range(M):
        for c in range(EC):
            wqk[m, c, :, :A] = Wq[m, c * P : (c + 1) * P, :]
            wqk[m, c, :, A:] = Wk[m, c * P : (c + 1) * P, :]
    wtb = np.empty((2 * M, EC, P, E), BFD)
    for j, (q, k) in enumerate(pairs):
        for c in range(EC):
            wtb[j, c] = Wt[q, k, c * P : (c + 1) * P, :]
    bqk = np.concatenate([ins["bq"], ins["bk"]], axis=1).astype(BFD)
    btr = np.stack([ins["bt"][q, k] for (q, k) in pairs]).astype(BFD)
    return {
        "Wqk_bf": wqk,
        "Wt_bf": wtb,
        "bqk_bf": np.ascontiguousarray(bqk),
        "btr_bf": np.ascontiguousarray(btr),
        "v_bf": ins["v"].astype(BFD),
        "eye_bf": np.eye(P, dtype=BFD),
        "gamma": ins["gamma"],
        "beta": ins["beta"],
    }


def kernel(**inputs) -> np.ndarray:
    ins = {
        k: np.ascontiguousarray(np.asarray(v, dtype=np.float32))
        for k, v in inputs.items()
    }
    x = ins["x"]
    assert x.shape == (M, B_FULL, E), x.shape
    fast_gb = bool(np.all(ins["gamma"] == 1.0) and np.all(ins["beta"] == 0.0))
    nc = _get_program(BC, fast_gb)

    shared = _prep_shared(ins)
    in_maps = []
    for i in range(N_CORES):
        m = dict(shared)
        m["x"] = np.ascontiguousarray(x[:, i * BC : (i + 1) * BC, :])
        in_maps.append(m)

    res = run_bass_kernel_spmd(nc, in_maps, core_ids=list(range(N_CORES)))
    out = np.concatenate([res.results[i]["out"] for i in range(N_CORES)], axis=0)
    return out


if __name__ == "__main__":
    rng = np.random.default_rng(0)
    ins = {
        "x": rng.standard_normal((M, B_FULL, E), dtype=np.float32),
        "Wq": (rng.standard_normal((M, E, A)) / np.sqrt(E)).astype(np.float32),
        "bq": (rng.standard_normal((M, A)) / np.sqrt(E)).astype(np.float32),
        "Wk": (rng.standard_normal((M, E, A)) / np.sqrt(E)).astype(np.float32),
        "bk": (rng.standard_normal((M, A)) / np.sqrt(E)).astype(np.float32),
        "v": (rng.standard_normal((M, A)) / np.sqrt(A)).astype(np.float32),
        "Wt": (rng.standard_normal((M, M, E, E)) / np.sqrt(E)).astype(np.float32),
        "bt": (rng.standard_normal((M, M, E)) / np.sqrt(E)).astype(np.float32),
        "gamma": np.ones((M * E,), np.float32),
        "beta": np.zeros((M * E,), np.float32),
    }
    out = kernel(**ins)
    print("out", out.shape, out.dtype)
